# revision 23
# baseline (speedup 1.0000x reference)
"""Gemma2 sliding-window attention (B=1, S=4096, HID=3584, 16 Q heads / 8 KV heads,
HD=256, window 2047, tanh softcap 50) on 8 Trainium2 NeuronCores.

Sharding: tensor-parallel over heads with partial-sum output unsharding.
Core c owns Q heads (2c, 2c+1) and KV head c.
  - Phase A: QKV projection computed transposed ([feature, token] layout) for Q/K
    (zero on-device transposes for the scores matmul); V in [token, feature] layout
    for the PV matmul. NeoX RoPE applied on the fly. w_qkv weights DMA'd in
    feature chunks so the first matmul chain starts early.
  - Phase B: scores computed transposed ([k, q] tiles); softmax without
    max-subtraction (tanh softcap bounds scores to +-50); boundary k-tiles are
    RAGGED (trimmed to the valid q-range) instead of fully masked, cutting ~15%
    of attention flops; remaining partial masks applied multiplicatively post-exp.
    The softmax denominator is accumulated on the (otherwise idle) Vector engine
    into an f32 tile, with a single ones-row matmul per (qb, head) — replacing
    the per-k-tile [1,512] denominator matmuls.
  - Phase C (fused per 512-token block, interleaved between head blocks): each
    core computes the PARTIAL output projection over its own 512 attention
    features for ALL 3584 output columns — same flops as a column slice of the
    full contraction, but no AllGather, no DRAM round-trip, and no barrier.
    Partial outputs (bf16) are summed on the host (the unshard step for
    partial-sum sharding).
"""

import sys

if "/opt/trn_rl_repo" not in sys.path:
    sys.path.insert(0, "/opt/trn_rl_repo")

import numpy as np
import ml_dtypes

import concourse.bass as bass
import concourse.tile as tile
from concourse import bacc, bass_isa, mybir
from concourse.bass_utils import run_bass_kernel_spmd

# Problem constants (hardcoded per harness contract)
S = 4096
HID = 3584
NH, NKV, HD = 16, 8, 256
Q_SIZE = NH * HD          # 4096
SCALE = 256.0 ** -0.5     # 1/16
SOFTCAP = 50.0
WINDOW = 2048 - 1         # 2047
THETA = 10000.0

N_CORES = 8
QK_F = 2 * HD + HD        # 768 per-core transposed-qk features: [q_h0, q_h1, k]
KO = HID // 128           # 28 contraction subtiles for the qkv projection
TT = S // 512             # 8 token tiles of 512
CT = HID // 512           # 7 output-column tiles of 512
F32 = mybir.dt.float32
BF16 = mybir.dt.bfloat16

# Boundary-tile diagonal offsets (q0 - 128*kt). Interior iff 128 <= off <= 1536.
MASK_OFFS = [-384, -256, -128, 0, 1664, 1792, 1920, 2048]

_NC_CACHE = {}


def _tile_plan(qb):
    """k-tiles for query block qb as (kt, qlo, qhi, mask_idx|None).

    Boundary tiles are trimmed to their valid q-range. The diagonal tile
    (off == 0, full width) is placed FIRST so the PV accumulation's start=True
    matmul covers all 512 columns before ragged start=False accumulations.
    """
    q0 = qb * 512
    plan = []
    for kt in range(max(0, 4 * qb - 16), 4 * qb + 4):
        off = q0 - 128 * kt
        if off == 0:
            e = (kt, 0, 512, MASK_OFFS.index(0))
        elif off < 0:                      # causal edge: -128/-256/-384
            e = (kt, -off, 512, MASK_OFFS.index(off))
        elif off >= 1792:                  # window edge: 1792/1920/2048
            e = (kt, 0, 2175 - off, MASK_OFFS.index(off))
        elif off == 1664:                  # 1-col ragged; keep full + mask
            e = (kt, 0, 512, MASK_OFFS.index(off))
        else:                              # interior
            e = (kt, 0, 512, None)
        kt, qlo, qhi, mi = e
        if mi is None:
            plan.append((kt, qlo, qhi, mi, 0, 0))
        else:
            # Columns where the tile's 128 keys straddle validity — only
            # this <=128-wide strip actually needs the mask multiply; all
            # other columns in [qlo, qhi) are fully valid.
            mlo = max(qlo, -off if off <= 0 else 2048 - off)
            mhi = min(qhi, mlo + 128)
            plan.append((kt, qlo, qhi, mi, mlo, mhi))
    diag = [e for e in plan if e[0] == 4 * qb]
    rest = [e for e in plan if e[0] != 4 * qb]
    return diag + rest


def _phase_a(nc, tc, qk_sb, v_sb, mask_sb, masks_r,
             hidT_r, wqkT_r, wvT_r, cosT, sinT):
    """QKV projection (transposed for Q/K, straight for V) + NeoX RoPE."""
    with (
        tc.tile_pool(name="wqk", bufs=1) as wqk_pool,
        tc.tile_pool(name="wv", bufs=1) as wv_pool,
        tc.tile_pool(name="hid", bufs=2) as hid_pool,
        tc.tile_pool(name="cs", bufs=2) as cs_pool,
        tc.tile_pool(name="rope", bufs=4) as rope_pool,
        tc.tile_pool(name="psA", bufs=3, space="PSUM") as psA,
        tc.tile_pool(name="psV", bufs=2, space="PSUM") as psV,
    ):
        # DMA issue order follows first-use order so the first matmul chain
        # starts after ~1.8MB of transfer instead of the full weight load:
        # qk-weight chunk 0, hid tile 0 in ko-quarters (the chain consumes ko
        # in order), chunk 1, rope tables, V weights (first V chain at
        # ~37us), the remaining qk chunks, then the attention masks.
        wqk_sb = wqk_pool.tile([128, KO, QK_F], BF16)
        nc.sync.dma_start(wqk_sb[:, :, bass.ts(0, 128)],
                          wqkT_r[:, :, bass.ts(0, 128)])
        hid0 = hid_pool.tile([128, KO, 512], BF16, name="hid_t")
        for kq in range(4):
            ksl = bass.ts(kq, 7)
            nc.sync.dma_start(hid0[:, ksl, :], hidT_r[:, ksl, 0:512])
        nc.sync.dma_start(wqk_sb[:, :, bass.ts(1, 128)],
                          wqkT_r[:, :, bass.ts(1, 128)])
        cos0 = cs_pool.tile([128, 512], F32, name="cos_t")
        nc.sync.dma_start(cos0, cosT[:, 0:512])
        sin0 = cs_pool.tile([128, 512], F32, name="sin_t")
        nc.sync.dma_start(sin0, sinT[:, 0:512])
        wv_sb = wv_pool.tile([128, KO, HD], BF16)
        nc.sync.dma_start(wv_sb, wvT_r)
        for f in (2, 3, 4, 5):
            fs = bass.ts(f, 128)
            nc.sync.dma_start(wqk_sb[:, :, fs], wqkT_r[:, :, fs])
        nc.sync.dma_start(mask_sb, masks_r)

        for tt in range(TT):
            tsl = bass.ts(tt, 512)
            if tt == 0:
                hid_t, cos_t, sin_t = hid0, cos0, sin0
            else:
                hid_t = hid_pool.tile([128, KO, 512], BF16, name="hid_t")
                nc.sync.dma_start(hid_t, hidT_r[:, :, tsl])
                cos_t = cs_pool.tile([128, 512], F32, name="cos_t")
                nc.sync.dma_start(cos_t, cosT[:, tsl])
                sin_t = cs_pool.tile([128, 512], F32, name="sin_t")
                nc.sync.dma_start(sin_t, sinT[:, tsl])

            if tt == TT - 1:
                # V first in the last tile: the phase-exit barrier then only
                # drains the short RoPE tail instead of the V psum copies.
                for ts4 in range(4):
                    ps_v = psV.tile([128, HD], F32, name="ps_v", tag="ps_v")
                    for ko in range(KO):
                        nc.tensor.matmul(
                            ps_v,
                            hid_t[:, ko, bass.ts(ts4, 128)],
                            wv_sb[:, ko, :],
                            start=(ko == 0), stop=(ko == KO - 1),
                        )
                    nc.scalar.copy(v_sb[:, tt * 4 + ts4, :], ps_v)

            for pair in range(3):
                ps_a = psA.tile([128, 512], F32, name="ps_qk", tag="ps_qk")
                for ko in range(KO):
                    nc.tensor.matmul(
                        ps_a,
                        wqk_sb[:, ko, bass.ts(2 * pair, 128)],
                        hid_t[:, ko, :],
                        start=(ko == 0), stop=(ko == KO - 1),
                    )
                ps_b = psA.tile([128, 512], F32, name="ps_qk2", tag="ps_qk")
                for ko in range(KO):
                    nc.tensor.matmul(
                        ps_b,
                        wqk_sb[:, ko, bass.ts(2 * pair + 1, 128)],
                        hid_t[:, ko, :],
                        start=(ko == 0), stop=(ko == KO - 1),
                    )
                # NeoX RoPE on the (x1, x2) pair, writing bf16 into qk_sb
                t1 = rope_pool.tile([128, 512], F32, name="rp1", tag="rp")
                t2 = rope_pool.tile([128, 512], F32, name="rp2", tag="rp")
                nc.vector.tensor_mul(t1, ps_a, cos_t)
                nc.vector.tensor_mul(t2, ps_b, sin_t)
                nc.vector.tensor_sub(qk_sb[:, 2 * pair, tsl], t1, t2)
                t3 = rope_pool.tile([128, 512], F32, name="rp3", tag="rp")
                t4 = rope_pool.tile([128, 512], F32, name="rp4", tag="rp")
                nc.vector.tensor_mul(t3, ps_b, cos_t)
                nc.vector.tensor_mul(t4, ps_a, sin_t)
                nc.vector.tensor_add(qk_sb[:, 2 * pair + 1, tsl], t3, t4)

            if tt < TT - 1:
                for ts4 in range(4):
                    ps_v = psV.tile([128, HD], F32, name="ps_v", tag="ps_v")
                    for ko in range(KO):
                        nc.tensor.matmul(
                            ps_v,
                            hid_t[:, ko, bass.ts(ts4, 128)],
                            wv_sb[:, ko, :],
                            start=(ko == 0), stop=(ko == KO - 1),
                        )
                    nc.scalar.copy(v_sb[:, tt * 4 + ts4, :], ps_v)


def _phase_bc(nc, tc, qk_sb, v_sb, mask_sb, ones_sb, wo_sb, out):
    """Fused attention + partial output projection.

    Per 512-token block qb: two head blocks (scores -> tanh/exp -> ragged
    mask -> PV accumulation, denominator accumulated on VectorE), then the
    partial o_proj for block qb-1 is interleaved between/after the head
    blocks so the Tensor engine never stalls on the softmax epilogue.
    """
    with (
        tc.tile_pool(name="probs", bufs=8) as probs_pool,
        tc.tile_pool(name="attn", bufs=2) as attn_pool,
        tc.tile_pool(name="accp", bufs=2) as acc_pool,
        tc.tile_pool(name="smalls", bufs=4) as small_pool,
        tc.tile_pool(name="outp", bufs=4) as out_pool,
        tc.tile_pool(name="psS", bufs=3, space="PSUM") as psS,
        tc.tile_pool(name="psO", bufs=1, space="PSUM") as psO,
        tc.tile_pool(name="psC", bufs=2, space="PSUM") as psC,
        tc.tile_pool(name="psD", bufs=1, space="PSUM") as psD,
    ):
        def emit_head(qb, h, attnT):
            q0 = qb * 512
            tiles = _tile_plan(qb)
            n = len(tiles)
            acc = acc_pool.tile([128, 512], F32, name="acc", tag="acc")
            nc.vector.memset(acc, 0.0)
            po0 = psO.tile([128, 512], F32, name="po0", tag="po0")
            po1 = psO.tile([128, 512], F32, name="po1", tag="po1")
            probs = {}

            def scores(i):
                kt, qlo, qhi, mi, mlo, mhi = tiles[i]
                w = qhi - qlo
                ksl = bass.ts(kt, 128)
                ps = psS.tile([128, 512], F32, name="ps_s", tag="ps_s")
                nc.tensor.matmul(
                    ps[:, :w], qk_sb[:, 4, ksl],
                    qk_sb[:, 2 * h, q0 + qlo:q0 + qhi],
                    start=True, stop=False,
                )
                nc.tensor.matmul(
                    ps[:, :w], qk_sb[:, 5, ksl],
                    qk_sb[:, 2 * h + 1, q0 + qlo:q0 + qhi],
                    start=False, stop=True,
                )
                pt = probs_pool.tile([128, 512], BF16, name="pt", tag="pt")
                nc.scalar.activation(
                    ps[:, :w], ps[:, :w], mybir.ActivationFunctionType.Tanh,
                    scale=SCALE / SOFTCAP,
                )
                nc.scalar.activation(
                    pt[:, :w], ps[:, :w], mybir.ActivationFunctionType.Exp,
                    scale=SOFTCAP,
                )
                if mi is not None:
                    nc.vector.tensor_mul(pt[:, mlo - qlo:mhi - qlo],
                                         pt[:, mlo - qlo:mhi - qlo],
                                         mask_sb[:, mi, mlo:mhi])
                nc.vector.tensor_add(acc[:, qlo:qhi], acc[:, qlo:qhi],
                                     pt[:, :w])
                probs[i] = pt

            def av(i):
                kt, qlo, qhi, mi, mlo, mhi = tiles[i]
                w = qhi - qlo
                pt = probs.pop(i)
                st, sp = (i == 0), (i == n - 1)
                nc.tensor.matmul(po0[:, qlo:qhi], v_sb[:, kt, 0:128],
                                 pt[:, :w], start=st, stop=sp,
                                 skip_group_check=True)
                nc.tensor.matmul(po1[:, qlo:qhi], v_sb[:, kt, 128:256],
                                 pt[:, :w], start=st, stop=sp,
                                 skip_group_check=True)

            LOOK = 3
            for i in range(min(LOOK, n)):
                scores(i)
            for i in range(n):
                if i + LOOK < n:
                    scores(i + LOOK)
                av(i)

            # Denominator: ones-row matmul over the vector-accumulated acc
            # (gpsimd PartitionAllReduce takes 3.5us — too slow), cast to bf16
            # first so the matmul runs at 1 cy/row instead of fp32's 4. Then a
            # fast approximate reciprocal (~18 bits, den is strictly
            # positive): the exact DVE reciprocal takes 3.3us and would block
            # the o_proj psum-drain copies queued behind it on vector.
            acc_bf = small_pool.tile([128, 512], BF16, name="acc_bf",
                                     tag="acc_bf")
            nc.vector.tensor_copy(acc_bf, acc)
            pden = psD.tile([1, 512], F32, name="pden", tag="pden")
            nc.tensor.matmul(pden, ones_sb, acc_bf, start=True, stop=True)
            recip = small_pool.tile([1, 512], F32, name="recip", tag="recip")
            nc.vector.reciprocal_approx_fast(recip, pden)
            rb = small_pool.tile([128, 512], F32, name="rb", tag="rb")
            nc.gpsimd.partition_broadcast(rb, recip)
            nc.vector.tensor_mul(attnT[:, 2 * h, :], po0, rb)
            nc.vector.tensor_mul(attnT[:, 2 * h + 1, :], po1, rb)

        def emit_c(qb, attnT, tbs):
            for tb in tbs:
                r0 = qb * 512 + tb * 128
                tbsl = bass.ts(tb, 128)
                for ct in range(CT):
                    csl = bass.ts(ct, 512)
                    ps = psC.tile([128, 512], F32, name="pc", tag="pc")
                    for fb in range(4):
                        nc.tensor.matmul(
                            ps, attnT[:, fb, tbsl], wo_sb[:, fb, csl],
                            start=(fb == 0), stop=(fb == 3),
                            skip_group_check=True,
                        )
                    ot = out_pool.tile([128, 512], BF16, name="ot", tag="ot")
                    nc.vector.tensor_copy(ot, ps)
                    nc.sync.dma_start(out[r0:r0 + 128, csl], ot)

        prev_attnT = None
        for qb in range(TT):
            attnT = attn_pool.tile([128, 4, 512], BF16, name="attnT",
                                   tag="attnT")
            emit_head(qb, 0, attnT)
            if prev_attnT is not None:
                emit_c(qb - 1, prev_attnT, (0, 1))
            emit_head(qb, 1, attnT)
            if prev_attnT is not None:
                emit_c(qb - 1, prev_attnT, (2, 3))
            prev_attnT = attnT
        emit_c(TT - 1, prev_attnT, (0, 1, 2, 3))


def build_nc():
    nc = bacc.Bacc()

    hidT = nc.declare_dram_parameter("hidT", [HID, S], BF16, isOutput=False)
    wqkT = nc.declare_dram_parameter("wqkT", [HID, QK_F], BF16, isOutput=False)
    wvT = nc.declare_dram_parameter("wvT", [HID, HD], BF16, isOutput=False)
    woT = nc.declare_dram_parameter("woT", [512, HID], BF16, isOutput=False)
    cosT = nc.declare_dram_parameter("cosT", [128, S], F32, isOutput=False)
    sinT = nc.declare_dram_parameter("sinT", [128, S], F32, isOutput=False)
    masks = nc.declare_dram_parameter("masks", [8, 128, 512], BF16, isOutput=False)
    out = nc.declare_dram_parameter("out", [S, HID], BF16, isOutput=True)

    hidT_r = hidT.rearrange("(ko p) t -> p ko t", p=128)
    wqkT_r = wqkT.rearrange("(ko p) f -> p ko f", p=128)
    wvT_r = wvT.rearrange("(ko p) d -> p ko d", p=128)
    woT_r = woT.rearrange("(fb p) c -> p fb c", p=128)
    masks_r = masks.rearrange("m p q -> p m q")

    with tile.TileContext(nc) as tc:
        with tc.tile_pool(name="persist", bufs=1) as persist:
            # live across phases A+BC
            qk_sb = persist.tile([128, 6, S], BF16)   # roped qT/kT rows: [h0x1,h0x2,h1x1,h1x2,kx1,kx2]
            v_sb = persist.tile([128, S // 128, HD], BF16)  # v in [token, d] layout
            mask_sb = persist.tile([128, 8, 512], BF16)  # DMA'd in _phase_a
            ones_sb = persist.tile([128, 1], BF16)
            nc.vector.memset(ones_sb, 1.0)

            _phase_a(nc, tc, qk_sb, v_sb, mask_sb, masks_r,
                     hidT_r, wqkT_r, wvT_r, cosT, sinT)

            with tc.tile_pool(name="wo", bufs=1) as wo_pool:
                # o-proj weights arrive during the first attention blocks
                wo_sb = wo_pool.tile([128, 4, HID], BF16)
                nc.sync.dma_start(wo_sb, woT_r)

                _phase_bc(nc, tc, qk_sb, v_sb, mask_sb, ones_sb, wo_sb, out)

    nc.compile()
    return nc


def get_nc():
    if "nc" not in _NC_CACHE:
        _NC_CACHE["nc"] = build_nc()
    return _NC_CACHE["nc"]


def prep_in_maps(inputs):
    bf16 = ml_dtypes.bfloat16
    hs = np.asarray(inputs["hidden_states"], dtype=np.float32)
    pos = np.asarray(inputs["position_ids"]).reshape(-1).astype(np.float64)
    w_qkv = np.asarray(inputs["w_qkv"], dtype=np.float32)
    w_o = np.asarray(inputs["w_o"], dtype=np.float32)

    hidT = np.ascontiguousarray(hs.reshape(S, HID).T).astype(bf16)

    inv_freq = 1.0 / (THETA ** (np.arange(HD // 2, dtype=np.float64) * 2.0 / HD))
    ang = inv_freq[:, None] * pos[None, :]
    cosT = np.cos(ang).astype(np.float32)
    sinT = np.sin(ang).astype(np.float32)

    kk = np.arange(128)[:, None]
    qq = np.arange(512)[None, :]
    masks = np.stack(
        [((qq - kk + o >= 0) & (qq - kk + o <= WINDOW)) for o in MASK_OFFS]
    ).astype(bf16)

    in_maps = []
    for c in range(N_CORES):
        wq = w_qkv[512 * c:512 * (c + 1)]
        wk = w_qkv[Q_SIZE + HD * c:Q_SIZE + HD * (c + 1)]
        wv = w_qkv[Q_SIZE + NKV * HD + HD * c:Q_SIZE + NKV * HD + HD * (c + 1)]
        wqkT = np.ascontiguousarray(np.concatenate([wq, wk], 0).T).astype(bf16)
        wvT = np.ascontiguousarray(wv.T).astype(bf16)
        # rows = this core's 512 local attention features, cols = all of HID
        woT = np.ascontiguousarray(w_o[:, 512 * c:512 * (c + 1)].T).astype(bf16)
        in_maps.append(
            dict(hidT=hidT, wqkT=wqkT, wvT=wvT, woT=woT,
                 cosT=cosT, sinT=sinT, masks=masks)
        )
    return in_maps


def run(inputs, **kwargs):
    nc = get_nc()
    in_maps = prep_in_maps(inputs)
    return run_bass_kernel_spmd(nc, in_maps, list(range(N_CORES)), **kwargs)


def gather(res):
    full = np.zeros((S, HID), dtype=np.float32)
    for c in range(N_CORES):
        full += np.asarray(res.results[c]["out"], dtype=np.float32)
    return full.reshape(1, S, HID)


def kernel(**inputs):
    return gather(run(inputs))


# revision 25
# speedup vs baseline: 1.0023x; 1.0023x over previous
"""Gemma2 sliding-window attention (B=1, S=4096, HID=3584, 16 Q heads / 8 KV heads,
HD=256, window 2047, tanh softcap 50) on 8 Trainium2 NeuronCores.

Sharding: tensor-parallel over heads with partial-sum output unsharding.
Core c owns Q heads (2c, 2c+1) and KV head c.
  - Phase A: QKV projection computed transposed ([feature, token] layout) for Q/K
    (zero on-device transposes for the scores matmul); V in [token, feature] layout
    for the PV matmul. NeoX RoPE applied on the fly. w_qkv weights DMA'd in
    feature chunks so the first matmul chain starts early.
  - Phase B: scores computed transposed ([k, q] tiles); softmax without
    max-subtraction (tanh softcap bounds scores to +-50); boundary k-tiles are
    RAGGED (trimmed to the valid q-range) instead of fully masked, cutting ~15%
    of attention flops; remaining partial masks applied multiplicatively post-exp.
    The softmax denominator is accumulated on the (otherwise idle) Vector engine
    into an f32 tile, with a single ones-row matmul per (qb, head) — replacing
    the per-k-tile [1,512] denominator matmuls.
  - Phase C (fused per 512-token block, interleaved between head blocks): each
    core computes the PARTIAL output projection over its own 512 attention
    features for ALL 3584 output columns — same flops as a column slice of the
    full contraction, but no AllGather, no DRAM round-trip, and no barrier.
    Partial outputs (bf16) are summed on the host (the unshard step for
    partial-sum sharding).
"""

import sys

if "/opt/trn_rl_repo" not in sys.path:
    sys.path.insert(0, "/opt/trn_rl_repo")

import numpy as np
import ml_dtypes

import concourse.bass as bass
import concourse.tile as tile
from concourse import bacc, bass_isa, mybir
from concourse.bass_utils import run_bass_kernel_spmd

# Problem constants (hardcoded per harness contract)
S = 4096
HID = 3584
NH, NKV, HD = 16, 8, 256
Q_SIZE = NH * HD          # 4096
SCALE = 256.0 ** -0.5     # 1/16
SOFTCAP = 50.0
WINDOW = 2048 - 1         # 2047
THETA = 10000.0

N_CORES = 8
QK_F = 2 * HD + HD        # 768 per-core transposed-qk features: [q_h0, q_h1, k]
KO = HID // 128           # 28 contraction subtiles for the qkv projection
TT = S // 512             # 8 token tiles of 512
CT = HID // 512           # 7 output-column tiles of 512
F32 = mybir.dt.float32
BF16 = mybir.dt.bfloat16

# Boundary-tile diagonal offsets (q0 - 128*kt). Interior iff 128 <= off <= 1536.
MASK_OFFS = [-384, -256, -128, 0, 1664, 1792, 1920, 2048]

_NC_CACHE = {}


def _tile_plan(qb):
    """k-tiles for query block qb as (kt, qlo, qhi, mask_idx|None).

    Boundary tiles are trimmed to their valid q-range. The diagonal tile
    (off == 0, full width) is placed FIRST so the PV accumulation's start=True
    matmul covers all 512 columns before ragged start=False accumulations.
    """
    q0 = qb * 512
    plan = []
    for kt in range(max(0, 4 * qb - 16), 4 * qb + 4):
        off = q0 - 128 * kt
        if off == 0:
            e = (kt, 0, 512, MASK_OFFS.index(0))
        elif off < 0:                      # causal edge: -128/-256/-384
            e = (kt, -off, 512, MASK_OFFS.index(off))
        elif off >= 1792:                  # window edge: 1792/1920/2048
            e = (kt, 0, 2175 - off, MASK_OFFS.index(off))
        elif off == 1664:                  # 1-col ragged; keep full + mask
            e = (kt, 0, 512, MASK_OFFS.index(off))
        else:                              # interior
            e = (kt, 0, 512, None)
        kt, qlo, qhi, mi = e
        if mi is None:
            plan.append((kt, qlo, qhi, mi, 0, 0))
        else:
            # Columns where the tile's 128 keys straddle validity — only
            # this <=128-wide strip actually needs the mask multiply; all
            # other columns in [qlo, qhi) are fully valid.
            mlo = max(qlo, -off if off <= 0 else 2048 - off)
            mhi = min(qhi, mlo + 128)
            plan.append((kt, qlo, qhi, mi, mlo, mhi))
    diag = [e for e in plan if e[0] == 4 * qb]
    rest = [e for e in plan if e[0] != 4 * qb]
    return diag + rest


def _phase_a(nc, tc, qk_sb, v_sb, mask_sb, masks_r,
             hidT_r, wqkT_r, wvT_r, cosT, sinT):
    """QKV projection (transposed for Q/K, straight for V) + NeoX RoPE."""
    with (
        tc.tile_pool(name="wqk", bufs=1) as wqk_pool,
        tc.tile_pool(name="wv", bufs=1) as wv_pool,
        tc.tile_pool(name="hid", bufs=2) as hid_pool,
        tc.tile_pool(name="cs", bufs=2) as cs_pool,
        tc.tile_pool(name="rope", bufs=4) as rope_pool,
        tc.tile_pool(name="psA", bufs=3, space="PSUM") as psA,
        tc.tile_pool(name="psV", bufs=2, space="PSUM") as psV,
    ):
        # DMA issue order follows first-use order so the first matmul chain
        # starts after ~1.8MB of transfer instead of the full weight load:
        # qk-weight chunk 0, hid tile 0 in ko-quarters (the chain consumes ko
        # in order), chunk 1, rope tables, V weights (first V chain at
        # ~37us), the remaining qk chunks, then the attention masks.
        wqk_sb = wqk_pool.tile([128, KO, QK_F], BF16)
        nc.sync.dma_start(wqk_sb[:, :, bass.ts(0, 128)],
                          wqkT_r[:, :, bass.ts(0, 128)])
        hid0 = hid_pool.tile([128, KO, 512], BF16, name="hid_t")
        for kq in range(4):
            ksl = bass.ts(kq, 7)
            nc.sync.dma_start(hid0[:, ksl, :], hidT_r[:, ksl, 0:512])
        nc.sync.dma_start(wqk_sb[:, :, bass.ts(1, 128)],
                          wqkT_r[:, :, bass.ts(1, 128)])
        cos0 = cs_pool.tile([128, 512], F32, name="cos_t")
        nc.sync.dma_start(cos0, cosT[:, 0:512])
        sin0 = cs_pool.tile([128, 512], F32, name="sin_t")
        nc.sync.dma_start(sin0, sinT[:, 0:512])
        wv_sb = wv_pool.tile([128, KO, HD], BF16)
        nc.sync.dma_start(wv_sb, wvT_r)
        for f in (2, 3, 4, 5):
            fs = bass.ts(f, 128)
            nc.sync.dma_start(wqk_sb[:, :, fs], wqkT_r[:, :, fs])
        nc.sync.dma_start(mask_sb, masks_r)

        for tt in range(TT):
            tsl = bass.ts(tt, 512)
            if tt == 0:
                hid_t, cos_t, sin_t = hid0, cos0, sin0
            else:
                hid_t = hid_pool.tile([128, KO, 512], BF16, name="hid_t")
                nc.sync.dma_start(hid_t, hidT_r[:, :, tsl])
                cos_t = cs_pool.tile([128, 512], F32, name="cos_t")
                nc.sync.dma_start(cos_t, cosT[:, tsl])
                sin_t = cs_pool.tile([128, 512], F32, name="sin_t")
                nc.sync.dma_start(sin_t, sinT[:, tsl])

            for pair in range(3):
                ps_a = psA.tile([128, 512], F32, name="ps_qk", tag="ps_qk")
                for ko in range(KO):
                    nc.tensor.matmul(
                        ps_a,
                        wqk_sb[:, ko, bass.ts(2 * pair, 128)],
                        hid_t[:, ko, :],
                        start=(ko == 0), stop=(ko == KO - 1),
                    )
                ps_b = psA.tile([128, 512], F32, name="ps_qk2", tag="ps_qk")
                for ko in range(KO):
                    nc.tensor.matmul(
                        ps_b,
                        wqk_sb[:, ko, bass.ts(2 * pair + 1, 128)],
                        hid_t[:, ko, :],
                        start=(ko == 0), stop=(ko == KO - 1),
                    )
                # NeoX RoPE on the (x1, x2) pair, writing bf16 into qk_sb
                t1 = rope_pool.tile([128, 512], F32, name="rp1", tag="rp")
                t2 = rope_pool.tile([128, 512], F32, name="rp2", tag="rp")
                nc.vector.tensor_mul(t1, ps_a, cos_t)
                nc.vector.tensor_mul(t2, ps_b, sin_t)
                nc.vector.tensor_sub(qk_sb[:, 2 * pair, tsl], t1, t2)
                t3 = rope_pool.tile([128, 512], F32, name="rp3", tag="rp")
                t4 = rope_pool.tile([128, 512], F32, name="rp4", tag="rp")
                nc.vector.tensor_mul(t3, ps_b, cos_t)
                nc.vector.tensor_mul(t4, ps_a, sin_t)
                nc.vector.tensor_add(qk_sb[:, 2 * pair + 1, tsl], t3, t4)

            for ts4 in range(4):
                ps_v = psV.tile([128, HD], F32, name="ps_v", tag="ps_v")
                for ko in range(KO):
                    nc.tensor.matmul(
                        ps_v,
                        hid_t[:, ko, bass.ts(ts4, 128)],
                        wv_sb[:, ko, :],
                        start=(ko == 0), stop=(ko == KO - 1),
                    )
                nc.scalar.copy(v_sb[:, tt * 4 + ts4, :], ps_v)


def _phase_bc(nc, tc, qk_sb, v_sb, mask_sb, ones_sb, wo_sb, out):
    """Fused attention + partial output projection.

    Per 512-token block qb: two head blocks (scores -> tanh/exp -> ragged
    mask -> PV accumulation, denominator accumulated on VectorE), then the
    partial o_proj for block qb-1 is interleaved between/after the head
    blocks so the Tensor engine never stalls on the softmax epilogue.
    """
    with (
        tc.tile_pool(name="probs", bufs=8) as probs_pool,
        tc.tile_pool(name="attn", bufs=2) as attn_pool,
        tc.tile_pool(name="accp", bufs=2) as acc_pool,
        tc.tile_pool(name="smalls", bufs=4) as small_pool,
        tc.tile_pool(name="outp", bufs=4) as out_pool,
        tc.tile_pool(name="psS", bufs=3, space="PSUM") as psS,
        tc.tile_pool(name="psO", bufs=1, space="PSUM") as psO,
        tc.tile_pool(name="psC", bufs=2, space="PSUM") as psC,
        tc.tile_pool(name="psD", bufs=1, space="PSUM") as psD,
    ):
        def emit_head(qb, h, attnT):
            q0 = qb * 512
            tiles = _tile_plan(qb)
            n = len(tiles)
            acc = acc_pool.tile([128, 512], F32, name="acc", tag="acc")
            nc.vector.memset(acc, 0.0)
            po0 = psO.tile([128, 512], F32, name="po0", tag="po0")
            po1 = psO.tile([128, 512], F32, name="po1", tag="po1")
            probs = {}

            def scores(i):
                kt, qlo, qhi, mi, mlo, mhi = tiles[i]
                w = qhi - qlo
                ksl = bass.ts(kt, 128)
                ps = psS.tile([128, 512], F32, name="ps_s", tag="ps_s")
                nc.tensor.matmul(
                    ps[:, :w], qk_sb[:, 4, ksl],
                    qk_sb[:, 2 * h, q0 + qlo:q0 + qhi],
                    start=True, stop=False,
                )
                nc.tensor.matmul(
                    ps[:, :w], qk_sb[:, 5, ksl],
                    qk_sb[:, 2 * h + 1, q0 + qlo:q0 + qhi],
                    start=False, stop=True,
                )
                pt = probs_pool.tile([128, 512], BF16, name="pt", tag="pt")
                nc.scalar.activation(
                    ps[:, :w], ps[:, :w], mybir.ActivationFunctionType.Tanh,
                    scale=SCALE / SOFTCAP,
                )
                nc.scalar.activation(
                    pt[:, :w], ps[:, :w], mybir.ActivationFunctionType.Exp,
                    scale=SOFTCAP,
                )
                if mi is not None:
                    nc.vector.tensor_mul(pt[:, mlo - qlo:mhi - qlo],
                                         pt[:, mlo - qlo:mhi - qlo],
                                         mask_sb[:, mi, mlo:mhi])
                nc.vector.tensor_add(acc[:, qlo:qhi], acc[:, qlo:qhi],
                                     pt[:, :w])
                probs[i] = pt

            def av(i):
                kt, qlo, qhi, mi, mlo, mhi = tiles[i]
                w = qhi - qlo
                pt = probs.pop(i)
                st, sp = (i == 0), (i == n - 1)
                nc.tensor.matmul(po0[:, qlo:qhi], v_sb[:, kt, 0:128],
                                 pt[:, :w], start=st, stop=sp,
                                 skip_group_check=True)
                nc.tensor.matmul(po1[:, qlo:qhi], v_sb[:, kt, 128:256],
                                 pt[:, :w], start=st, stop=sp,
                                 skip_group_check=True)

            LOOK = 3
            for i in range(min(LOOK, n)):
                scores(i)
            for i in range(n):
                if i + LOOK < n:
                    scores(i + LOOK)
                av(i)

            # Denominator: ones-row matmul over the vector-accumulated acc
            # (gpsimd PartitionAllReduce takes 3.5us — too slow), cast to bf16
            # first so the matmul runs at 1 cy/row instead of fp32's 4. Then a
            # fast approximate reciprocal (~18 bits, den is strictly
            # positive): the exact DVE reciprocal takes 3.3us and would block
            # the o_proj psum-drain copies queued behind it on vector.
            acc_bf = small_pool.tile([128, 512], BF16, name="acc_bf",
                                     tag="acc_bf")
            nc.vector.tensor_copy(acc_bf, acc)
            pden = psD.tile([1, 512], F32, name="pden", tag="pden")
            nc.tensor.matmul(pden, ones_sb, acc_bf, start=True, stop=True)
            recip = small_pool.tile([1, 512], F32, name="recip", tag="recip")
            nc.vector.reciprocal_approx_fast(recip, pden)
            rb = small_pool.tile([128, 512], F32, name="rb", tag="rb")
            nc.gpsimd.partition_broadcast(rb, recip)
            nc.vector.tensor_mul(attnT[:, 2 * h, :], po0, rb)
            nc.vector.tensor_mul(attnT[:, 2 * h + 1, :], po1, rb)

        def emit_c(qb, attnT, tbs):
            for tb in tbs:
                r0 = qb * 512 + tb * 128
                tbsl = bass.ts(tb, 128)
                for ct in range(CT):
                    csl = bass.ts(ct, 512)
                    ps = psC.tile([128, 512], F32, name="pc", tag="pc")
                    for fb in range(4):
                        nc.tensor.matmul(
                            ps, attnT[:, fb, tbsl], wo_sb[:, fb, csl],
                            start=(fb == 0), stop=(fb == 3),
                            skip_group_check=True,
                        )
                    ot = out_pool.tile([128, 512], BF16, name="ot", tag="ot")
                    nc.vector.tensor_copy(ot, ps)
                    nc.sync.dma_start(out[r0:r0 + 128, csl], ot)

        prev_attnT = None
        for qb in range(TT):
            attnT = attn_pool.tile([128, 4, 512], BF16, name="attnT",
                                   tag="attnT")
            emit_head(qb, 0, attnT)
            if prev_attnT is not None:
                emit_c(qb - 1, prev_attnT, (0, 1))
            emit_head(qb, 1, attnT)
            if prev_attnT is not None:
                emit_c(qb - 1, prev_attnT, (2, 3))
            prev_attnT = attnT
        emit_c(TT - 1, prev_attnT, (0, 1, 2, 3))


def build_nc():
    nc = bacc.Bacc()

    hidT = nc.declare_dram_parameter("hidT", [HID, S], BF16, isOutput=False)
    wqkT = nc.declare_dram_parameter("wqkT", [HID, QK_F], BF16, isOutput=False)
    wvT = nc.declare_dram_parameter("wvT", [HID, HD], BF16, isOutput=False)
    woT = nc.declare_dram_parameter("woT", [512, HID], BF16, isOutput=False)
    cosT = nc.declare_dram_parameter("cosT", [128, S], F32, isOutput=False)
    sinT = nc.declare_dram_parameter("sinT", [128, S], F32, isOutput=False)
    masks = nc.declare_dram_parameter("masks", [8, 128, 512], BF16, isOutput=False)
    out = nc.declare_dram_parameter("out", [S, HID], BF16, isOutput=True)

    hidT_r = hidT.rearrange("(ko p) t -> p ko t", p=128)
    wqkT_r = wqkT.rearrange("(ko p) f -> p ko f", p=128)
    wvT_r = wvT.rearrange("(ko p) d -> p ko d", p=128)
    woT_r = woT.rearrange("(fb p) c -> p fb c", p=128)
    masks_r = masks.rearrange("m p q -> p m q")

    with tile.TileContext(nc) as tc:
        with tc.tile_pool(name="persist", bufs=1) as persist:
            # live across phases A+BC
            qk_sb = persist.tile([128, 6, S], BF16)   # roped qT/kT rows: [h0x1,h0x2,h1x1,h1x2,kx1,kx2]
            v_sb = persist.tile([128, S // 128, HD], BF16)  # v in [token, d] layout
            mask_sb = persist.tile([128, 8, 512], BF16)  # DMA'd in _phase_a
            ones_sb = persist.tile([128, 1], BF16)
            nc.vector.memset(ones_sb, 1.0)

            _phase_a(nc, tc, qk_sb, v_sb, mask_sb, masks_r,
                     hidT_r, wqkT_r, wvT_r, cosT, sinT)

            with tc.tile_pool(name="wo", bufs=1) as wo_pool:
                # o-proj weights arrive during the first attention blocks
                wo_sb = wo_pool.tile([128, 4, HID], BF16)
                nc.sync.dma_start(wo_sb, woT_r)

                _phase_bc(nc, tc, qk_sb, v_sb, mask_sb, ones_sb, wo_sb, out)

    nc.compile()
    return nc


def get_nc():
    if "nc" not in _NC_CACHE:
        _NC_CACHE["nc"] = build_nc()
    return _NC_CACHE["nc"]


def prep_in_maps(inputs):
    bf16 = ml_dtypes.bfloat16
    hs = np.asarray(inputs["hidden_states"], dtype=np.float32)
    pos = np.asarray(inputs["position_ids"]).reshape(-1).astype(np.float64)
    w_qkv = np.asarray(inputs["w_qkv"], dtype=np.float32)
    w_o = np.asarray(inputs["w_o"], dtype=np.float32)

    hidT = np.ascontiguousarray(hs.reshape(S, HID).T).astype(bf16)

    inv_freq = 1.0 / (THETA ** (np.arange(HD // 2, dtype=np.float64) * 2.0 / HD))
    ang = inv_freq[:, None] * pos[None, :]
    cosT = np.cos(ang).astype(np.float32)
    sinT = np.sin(ang).astype(np.float32)

    kk = np.arange(128)[:, None]
    qq = np.arange(512)[None, :]
    masks = np.stack(
        [((qq - kk + o >= 0) & (qq - kk + o <= WINDOW)) for o in MASK_OFFS]
    ).astype(bf16)

    in_maps = []
    for c in range(N_CORES):
        wq = w_qkv[512 * c:512 * (c + 1)]
        wk = w_qkv[Q_SIZE + HD * c:Q_SIZE + HD * (c + 1)]
        wv = w_qkv[Q_SIZE + NKV * HD + HD * c:Q_SIZE + NKV * HD + HD * (c + 1)]
        wqkT = np.ascontiguousarray(np.concatenate([wq, wk], 0).T).astype(bf16)
        wvT = np.ascontiguousarray(wv.T).astype(bf16)
        # rows = this core's 512 local attention features, cols = all of HID
        woT = np.ascontiguousarray(w_o[:, 512 * c:512 * (c + 1)].T).astype(bf16)
        in_maps.append(
            dict(hidT=hidT, wqkT=wqkT, wvT=wvT, woT=woT,
                 cosT=cosT, sinT=sinT, masks=masks)
        )
    return in_maps


def run(inputs, **kwargs):
    nc = get_nc()
    in_maps = prep_in_maps(inputs)
    return run_bass_kernel_spmd(nc, in_maps, list(range(N_CORES)), **kwargs)


def gather(res):
    full = np.zeros((S, HID), dtype=np.float32)
    for c in range(N_CORES):
        full += np.asarray(res.results[c]["out"], dtype=np.float32)
    return full.reshape(1, S, HID)


def kernel(**inputs):
    return gather(run(inputs))
